# revision 5
# baseline (speedup 1.0000x reference)
"""DCN cross-network forward on 8 Trainium2 NeuronCores — 16-bit pipeline.

Reference computation (LAYER_NUM=4, INPUT_DIM=1024, BATCH=16384):
    x0 = x
    for i in range(4):
        s  = xi @ w[i]                      # [B] per-row scalar
        xi = x0 * s[:, None] + b[i] + xi

Algebraic collapse: every layer adds a per-row multiple of x0 plus a
constant vector, so
    x_i = alpha_i * x0 + C_i,   C_i = sum_{j<i} b[j]
    t_i = x0 . w[i]             (per-row scalars, the only real compute)
    k_i = C_i . w[i]            (host-computable scalar constants)
    alpha_{i+1} = alpha_i * (1 + t_i) + k_i,  alpha_0 = 1
    out = alpha_4 * x0 + C_4
which reads x exactly once and writes out exactly once.  The rel-err
gate (2e-2) leaves room for a 16-bit device pipeline:
  - host casts x to fp16 (dot rel err ~5e-4)
  - device reads fp16, computes t_i and alpha (fp32), writes
    out = alpha*x in bf16 (alpha reaches ~2e7; fp16 out would overflow)
  - host upcasts to fp32 and adds C_4
This halves HBM traffic vs fp32: ~4.2MB in + ~4.2MB out per core — the
memory roofline is ~24us/core.

Device pipeline per 128-row tile (16 tiles/core, 4 groups of 4):
  - TensorE: 8 fp16 chunk transposes (is_transpose keeps fp16 in PSUM,
    ~107ns cadence), then 8 accumulating dot matmuls vs w^T [128,4]
    chunks (~26ns cadence, LDWEIGHTS hidden) -> t in PSUM [128,4] fp32.
  - ScalarE: PSUM->SBUF copy of the transposed chunks (dot matmul lhsT
    must live in SBUF).
  - DVE: batched per-group alpha recurrence (6 strided ops on [128,4])
    and the out-scale (tensor_scalar 16-bit fast mode).
Host-side layout shuffle gives every DMA 128 partitions x 8KB
contiguous descriptors; identity/w^T consts are issued first so the PE
can start at ~3us.

Sharding: data-parallel over batch; each of 8 cores gets [2048, 1024].
"""

import sys

import numpy as np

sys.path.insert(0, "/opt/trn_rl_repo")

BATCH = 16384
D = 1024
L = 4
NCORES = 8
SHARD = BATCH // NCORES  # 2048
P = 128
NT = SHARD // P          # 16 row-tiles per core
NCH = D // P             # 8 contraction chunks
NG = NT // 4             # 4 groups of 4 tiles

_build_cache: dict = {}


def _build_program(k1: float, k2: float, k3: float):
    """Build (and compile) the SPMD Bass program for one core's shard."""
    import concourse.bacc as bacc
    import concourse.mybir as mybir
    import concourse.tile as tile
    f16 = mybir.dt.float16
    bf16 = mybir.dt.bfloat16
    f32 = mybir.dt.float32
    mult = mybir.AluOpType.mult
    add = mybir.AluOpType.add

    nc = bacc.Bacc("TRN2", target_bir_lowering=False, debug=False)

    # host pre-shuffled layout: x[s, p, h, :] = row (s*512 + h*128 + p)
    x = nc.dram_tensor("x", [NG, P, 4, D], f16, kind="ExternalInput").ap()
    wtd = nc.dram_tensor("wtd", [P, NCH, L], f16, kind="ExternalInput").ap()
    idd = nc.dram_tensor("idd", [P, P], f16, kind="ExternalInput").ap()
    out = nc.dram_tensor("out", [NG, P, 4, D], bf16, kind="ExternalOutput").ap()

    with tile.TileContext(nc) as tc:
        with (
            tc.tile_pool(name="consts", bufs=1) as cpool,
            tc.tile_pool(name="xin", bufs=4) as xpool,
            tc.tile_pool(name="xtr", bufs=3) as xtpool,
            tc.tile_pool(name="small", bufs=8) as spool,
            tc.tile_pool(name="outp", bufs=3) as opool,
            tc.tile_pool(name="ps_tr", bufs=3, space="PSUM") as pst,
            tc.tile_pool(name="ps_t", bufs=2, space="PSUM") as psv,
        ):
            ident = cpool.tile([P, P], f16)
            wt_sb = cpool.tile([P, NCH, L], f16)
            with tc.high_priority(offset=1000):
                nc.sync.dma_start(out=ident[:], in_=idd)
                nc.sync.dma_start(out=wt_sb[:], in_=wtd)

            xt2 = None
            o2 = None
            tps = None
            tvg = None
            # ScalarE takes the first SC_CH transposed chunks, DVE the rest
            SC_CH = 6
            for j in range(NT):
                s_idx, h = j // 4, j % 4
                if h == 0:
                    xt2 = xpool.tile([P, 4, D], f16, tag="x")
                    with tc.high_priority(offset=15):
                        if s_idx == 0 or s_idx == NG - 1:
                            # fine-grained first/last group: tighter pipeline
                            for hh in range(4):
                                nc.sync.dma_start(
                                    out=xt2[:, hh, :], in_=x[s_idx, :, hh, :]
                                )
                        else:
                            nc.sync.dma_start(out=xt2[:], in_=x[s_idx])
                    o2 = opool.tile([P, 4, D], bf16, tag="o")
                    tps = psv.tile([P, 4, L], f32, tag="tps")
                xt = xt2[:, h, :]

                # --- TensorE: transpose chunks, then dot vs w^T chunks ---
                xtp = pst.tile([P, NCH, P], f16, tag="xtp")
                for c in range(NCH):
                    nc.tensor.transpose(
                        xtp[:, c, :], xt[:, c * P : (c + 1) * P], ident[:]
                    )
                xts = xtpool.tile([P, NCH, P], f16, tag="xts")
                nc.scalar.copy(
                    out=xts[:, :SC_CH, :], in_=xtp[:, :SC_CH, :]
                )
                nc.vector.tensor_copy(xts[:, SC_CH:, :], xtp[:, SC_CH:, :])
                for c in range(NCH):
                    nc.tensor.matmul(
                        tps[:, h, :],
                        lhsT=xts[:, c, :],
                        rhs=wt_sb[:, c, :],
                        start=(c == 0),
                        stop=(c == NCH - 1),
                    )

                if h == 3:
                    # --- batched recurrence for the group (raw-t form) ---
                    # alpha4 = ((((1+t0)(1+t1)+k1)(1+t2)+k2)(1+t3))+k3
                    tvg = spool.tile([P, 4, L], f32, tag="tv")
                    nc.vector.tensor_copy(tvg[:], tps[:])
                    t0 = tvg[:, :, 0]
                    t1 = tvg[:, :, 1]
                    t2 = tvg[:, :, 2]
                    t3 = tvg[:, :, 3]
                    u = spool.tile([P, 4], f32, tag="u")
                    nc.vector.scalar_tensor_tensor(
                        out=u[:], in0=t1, scalar=1.0, in1=t0, op0=add, op1=mult
                    )
                    al2 = spool.tile([P, 4], f32, tag="al2")
                    nc.vector.scalar_tensor_tensor(
                        out=al2[:], in0=u[:], scalar=1.0 + k1, in1=t1,
                        op0=add, op1=add,
                    )
                    w3 = spool.tile([P, 4], f32, tag="w3")
                    nc.vector.scalar_tensor_tensor(
                        out=w3[:], in0=t2, scalar=1.0, in1=al2[:],
                        op0=add, op1=mult,
                    )
                    z = spool.tile([P, 4], f32, tag="z")
                    nc.vector.scalar_tensor_tensor(
                        out=z[:], in0=t3, scalar=1.0, in1=w3[:],
                        op0=add, op1=mult,
                    )
                    y = spool.tile([P, 4], f32, tag="y")
                    nc.vector.scalar_tensor_tensor(
                        out=y[:], in0=t3, scalar=k2, in1=z[:],
                        op0=mult, op1=add,
                    )
                    a4g = spool.tile([P, 4], f32, tag="a4")
                    nc.vector.tensor_scalar(
                        a4g[:], y[:], k2 + k3, None, op0=add
                    )
                    # --- out = x * alpha (DVE 16-bit fast mode) ---
                    for hh in range(4):
                        nc.vector.tensor_scalar_mul(
                            o2[:, hh, :], xt2[:, hh, :], a4g[:, hh : hh + 1]
                        )
                    # outputs ride the second HWDGE queue (ACT engine) so
                    # they never queue behind input issues on Sync
                    if s_idx == NG - 1:
                        # fine-grained last group: drain the tail sooner
                        for hh in range(4):
                            nc.scalar.dma_start(
                                out=out[s_idx, :, hh, :], in_=o2[:, hh, :]
                            )
                    else:
                        nc.scalar.dma_start(out=out[s_idx], in_=o2[:])

    nc.compile()
    return nc


def _shuffle(x16):
    """[2048, 1024] -> [NG, P, 4, D] with x'[s, p, h] = x[s*512 + h*128 + p]."""
    return np.ascontiguousarray(
        x16.reshape(NG, 4, P, D).transpose(0, 2, 1, 3)
    )


def _make_in_maps(x16, W16):
    """Per-core input maps; x16/W16 are fp16 C-contiguous [B,D] and [L,D]."""
    # wt: w^T chunks, wt[p, c, i] = w[i, c*128+p]
    wt = np.ascontiguousarray(W16.reshape(L, NCH, P).transpose(2, 1, 0))
    ident = np.eye(P, dtype=np.float16)
    return [
        {
            "x": _shuffle(x16[c * SHARD : (c + 1) * SHARD]),
            "wtd": wt,
            "idd": ident,
        }
        for c in range(NCORES)
    ]


def kernel(x, cross_weights, cross_bias):
    from concourse.bass_utils import run_bass_kernel_spmd

    x = np.asarray(x, dtype=np.float32)
    W = np.asarray(cross_weights, dtype=np.float32)
    Bb = np.asarray(cross_bias, dtype=np.float32)
    assert x.shape == (BATCH, D) and W.shape == (L, D) and Bb.shape == (L, D)

    # host-side scalar constants k_i = C_i . w_i with C_i = sum_{j<i} b_j
    C = np.zeros(D, dtype=np.float32)
    ks = []
    for i in range(L):
        ks.append(float(C @ W[i]))
        C = C + Bb[i]
    # ks[0] == 0 always (C_0 = 0); bake the other three
    k1, k2, k3 = ks[1], ks[2], ks[3]

    key = (k1, k2, k3)
    nc = _build_cache.get(key)
    if nc is None:
        nc = _build_program(k1, k2, k3)
        _build_cache[key] = nc

    x16 = np.ascontiguousarray(x.astype(np.float16))
    W16 = np.ascontiguousarray(W.astype(np.float16))
    in_maps = _make_in_maps(x16, W16)
    res = run_bass_kernel_spmd(nc, in_maps, list(range(NCORES)))
    # un-shuffle: out'[s, p, h] -> row (s*512 + h*128 + p), upcast, add C4
    full = np.empty((BATCH, D), dtype=np.float32)
    for c in range(NCORES):
        oc = np.asarray(res.results[c]["out"])  # [NG, P, 4, D] bf16
        full[c * SHARD : (c + 1) * SHARD] = (
            oc.transpose(0, 2, 1, 3).reshape(SHARD, D).astype(np.float32)
        )
    full += C[None, :]  # C4 broadcast-add on host
    return full


# revision 10
# speedup vs baseline: 1.1137x; 1.1137x over previous
"""DCN cross-network forward on 8 Trainium2 NeuronCores — 16-bit pipeline.

Reference computation (LAYER_NUM=4, INPUT_DIM=1024, BATCH=16384):
    x0 = x
    for i in range(4):
        s  = xi @ w[i]                      # [B] per-row scalar
        xi = x0 * s[:, None] + b[i] + xi

Algebraic collapse: every layer adds a per-row multiple of x0 plus a
constant vector, so
    x_i = alpha_i * x0 + C_i,   C_i = sum_{j<i} b[j]
    t_i = x0 . w[i]             (per-row scalars, the only real compute)
    k_i = C_i . w[i]            (host-computable scalar constants)
    alpha_{i+1} = alpha_i * (1 + t_i) + k_i,  alpha_0 = 1
    out = alpha_4 * x0 + C_4
which reads x exactly once and writes out exactly once.  The rel-err
gate (2e-2) leaves room for a 16-bit device pipeline:
  - host casts x to fp16 (dot rel err ~5e-4)
  - device reads fp16, computes t_i and alpha (fp32), writes
    out = alpha*x in bf16 (alpha reaches ~2e7; fp16 out would overflow)
  - host upcasts to fp32 and adds C_4
This halves HBM traffic vs fp32: ~4.2MB in + ~4.2MB out per core — the
memory roofline is ~24us/core.

Device pipeline per 128-row tile (16 tiles/core, 4 groups of 4):
  - TensorE: 8 fp16 chunk transposes (is_transpose keeps fp16 in PSUM,
    ~107ns cadence), then 8 accumulating dot matmuls vs w^T [128,4]
    chunks (~26ns cadence, LDWEIGHTS hidden) -> t in PSUM [128,4] fp32.
  - ScalarE: PSUM->SBUF copy of the transposed chunks (dot matmul lhsT
    must live in SBUF).
  - DVE: batched per-group alpha recurrence (6 strided ops on [128,4])
    and the out-scale (tensor_scalar 16-bit fast mode).
Host-side layout shuffle gives every DMA 128 partitions x 8KB
contiguous descriptors; identity/w^T consts are issued first so the PE
can start at ~3us.

Sharding: data-parallel over batch; each of 8 cores gets [2048, 1024].
"""

import sys

import numpy as np

sys.path.insert(0, "/opt/trn_rl_repo")

BATCH = 16384
D = 1024
L = 4
NCORES = 8
SHARD = BATCH // NCORES  # 2048
P = 128
NT = SHARD // P          # 16 row-tiles per core
NCH = D // P             # 8 contraction chunks
NG = NT // 4             # 4 groups of 4 tiles

_build_cache: dict = {}


def _build_program(k1: float, k2: float, k3: float):
    """Build (and compile) the SPMD Bass program for one core's shard."""
    import concourse.bacc as bacc
    import concourse.mybir as mybir
    import concourse.tile as tile
    f16 = mybir.dt.float16
    bf16 = mybir.dt.bfloat16
    f32 = mybir.dt.float32
    mult = mybir.AluOpType.mult
    add = mybir.AluOpType.add

    nc = bacc.Bacc("TRN2", target_bir_lowering=False, debug=False)

    # host pre-shuffled layout: x[s, p, h, :] = row (s*512 + h*128 + p)
    x = nc.dram_tensor("x", [NG, P, 4, D], f16, kind="ExternalInput").ap()
    # consts packed in one tensor: [:, :P] identity, [:, P:] w^T chunks
    cst = nc.dram_tensor(
        "cst", [P, P + NCH * L], f16, kind="ExternalInput"
    ).ap()
    out = nc.dram_tensor("out", [NG, P, 4, D], bf16, kind="ExternalOutput").ap()

    with tile.TileContext(nc) as tc:
        with (
            tc.tile_pool(name="consts", bufs=1) as cpool,
            tc.tile_pool(name="xin", bufs=4) as xpool,
            tc.tile_pool(name="xtr", bufs=3) as xtpool,
            tc.tile_pool(name="small", bufs=8) as spool,
            tc.tile_pool(name="outp", bufs=3) as opool,
            tc.tile_pool(name="ps_tr", bufs=3, space="PSUM") as pst,
            tc.tile_pool(name="ps_t", bufs=2, space="PSUM") as psv,
        ):
            csts = cpool.tile([P, P + NCH * L], f16)
            with tc.high_priority(offset=1000):
                nc.sync.dma_start(out=csts[:], in_=cst)
            ident = csts[:, :P]
            wt_sb = csts[:, P:].rearrange("p (c l) -> p c l", c=NCH, l=L)

            xt2 = None
            o2 = None
            tps = None
            tvg = None
            # ScalarE takes the first SC_CH transposed chunks, DVE the rest
            SC_CH = 6
            for j in range(NT):
                s_idx, h = j // 4, j % 4
                if h == 0:
                    xt2 = xpool.tile([P, 4, D], f16, tag="x")
                    with tc.high_priority(offset=15):
                        if s_idx == 0 or s_idx == NG - 1:
                            # fine-grained first/last group: tighter pipeline
                            for hh in range(4):
                                nc.sync.dma_start(
                                    out=xt2[:, hh, :], in_=x[s_idx, :, hh, :]
                                )
                        else:
                            nc.sync.dma_start(out=xt2[:], in_=x[s_idx])
                    o2 = opool.tile([P, 4, D], bf16, tag="o")
                    tps = psv.tile([P, 4, L], f32, tag="tps")
                xt = xt2[:, h, :]

                # --- TensorE: transpose chunks, then dot vs w^T chunks ---
                xtp = pst.tile([P, NCH, P], f16, tag="xtp")
                for c in range(NCH):
                    nc.tensor.transpose(
                        xtp[:, c, :], xt[:, c * P : (c + 1) * P], ident
                    )
                xts = xtpool.tile([P, NCH, P], f16, tag="xts")
                nc.scalar.copy(
                    out=xts[:, :SC_CH, :], in_=xtp[:, :SC_CH, :]
                )
                nc.vector.tensor_copy(xts[:, SC_CH:, :], xtp[:, SC_CH:, :])
                for c in range(NCH):
                    nc.tensor.matmul(
                        tps[:, h, :],
                        lhsT=xts[:, c, :],
                        rhs=wt_sb[:, c, :],
                        start=(c == 0),
                        stop=(c == NCH - 1),
                    )

                if h == 3:
                    # --- batched recurrence for the group (raw-t form) ---
                    # alpha4 = ((((1+t0)(1+t1)+k1)(1+t2)+k2)(1+t3))+k3
                    tvg = spool.tile([P, 4, L], f32, tag="tv")
                    nc.vector.tensor_copy(tvg[:], tps[:])
                    t0 = tvg[:, :, 0]
                    t1 = tvg[:, :, 1]
                    t2 = tvg[:, :, 2]
                    t3 = tvg[:, :, 3]
                    u = spool.tile([P, 4], f32, tag="u")
                    nc.vector.scalar_tensor_tensor(
                        out=u[:], in0=t1, scalar=1.0, in1=t0, op0=add, op1=mult
                    )
                    al2 = spool.tile([P, 4], f32, tag="al2")
                    nc.vector.scalar_tensor_tensor(
                        out=al2[:], in0=u[:], scalar=1.0 + k1, in1=t1,
                        op0=add, op1=add,
                    )
                    w3 = spool.tile([P, 4], f32, tag="w3")
                    nc.vector.scalar_tensor_tensor(
                        out=w3[:], in0=t2, scalar=1.0, in1=al2[:],
                        op0=add, op1=mult,
                    )
                    z = spool.tile([P, 4], f32, tag="z")
                    nc.vector.scalar_tensor_tensor(
                        out=z[:], in0=t3, scalar=1.0, in1=w3[:],
                        op0=add, op1=mult,
                    )
                    y = spool.tile([P, 4], f32, tag="y")
                    nc.vector.scalar_tensor_tensor(
                        out=y[:], in0=t3, scalar=k2, in1=z[:],
                        op0=mult, op1=add,
                    )
                    a4g = spool.tile([P, 4], f32, tag="a4")
                    nc.vector.tensor_scalar(
                        a4g[:], y[:], k2 + k3, None, op0=add
                    )
                    # --- out = x * alpha (DVE 16-bit fast mode) ---
                    for hh in range(4):
                        nc.vector.tensor_scalar_mul(
                            o2[:, hh, :], xt2[:, hh, :], a4g[:, hh : hh + 1]
                        )
                    if s_idx == NG - 1:
                        # fine-grained last group: drain the tail sooner
                        for hh in range(4):
                            nc.sync.dma_start(
                                out=out[s_idx, :, hh, :], in_=o2[:, hh, :]
                            )
                    else:
                        nc.sync.dma_start(out=out[s_idx], in_=o2[:])

    nc.compile()
    return nc


def _shuffle(x16):
    """[2048, 1024] -> [NG, P, 4, D] with x'[s, p, h] = x[s*512 + h*128 + p]."""
    return np.ascontiguousarray(
        x16.reshape(NG, 4, P, D).transpose(0, 2, 1, 3)
    )


def _make_in_maps(x16, W16):
    """Per-core input maps; x16/W16 are fp16 C-contiguous [B,D] and [L,D]."""
    # consts: [:, :P] identity, [:, P:] w^T chunks wt[p, c, i] = w[i, c*128+p]
    cst = np.empty((P, P + NCH * L), dtype=np.float16)
    cst[:, :P] = np.eye(P, dtype=np.float16)
    cst[:, P:] = W16.reshape(L, NCH, P).transpose(2, 1, 0).reshape(P, NCH * L)
    return [
        {
            "x": _shuffle(x16[c * SHARD : (c + 1) * SHARD]),
            "cst": cst,
        }
        for c in range(NCORES)
    ]


def kernel(x, cross_weights, cross_bias):
    from concourse.bass_utils import run_bass_kernel_spmd

    x = np.asarray(x, dtype=np.float32)
    W = np.asarray(cross_weights, dtype=np.float32)
    Bb = np.asarray(cross_bias, dtype=np.float32)
    assert x.shape == (BATCH, D) and W.shape == (L, D) and Bb.shape == (L, D)

    # host-side scalar constants k_i = C_i . w_i with C_i = sum_{j<i} b_j
    C = np.zeros(D, dtype=np.float32)
    ks = []
    for i in range(L):
        ks.append(float(C @ W[i]))
        C = C + Bb[i]
    # ks[0] == 0 always (C_0 = 0); bake the other three
    k1, k2, k3 = ks[1], ks[2], ks[3]

    key = (k1, k2, k3)
    nc = _build_cache.get(key)
    if nc is None:
        nc = _build_program(k1, k2, k3)
        _build_cache[key] = nc

    x16 = np.ascontiguousarray(x.astype(np.float16))
    W16 = np.ascontiguousarray(W.astype(np.float16))
    in_maps = _make_in_maps(x16, W16)
    res = run_bass_kernel_spmd(nc, in_maps, list(range(NCORES)))
    # un-shuffle: out'[s, p, h] -> row (s*512 + h*128 + p), upcast, add C4
    full = np.empty((BATCH, D), dtype=np.float32)
    for c in range(NCORES):
        oc = np.asarray(res.results[c]["out"])  # [NG, P, 4, D] bf16
        full[c * SHARD : (c + 1) * SHARD] = (
            oc.transpose(0, 2, 1, 3).reshape(SHARD, D).astype(np.float32)
        )
    full += C[None, :]  # C4 broadcast-add on host
    return full
